# revision 33
# baseline (speedup 1.0000x reference)
"""Multi-head attention (B=4,S=2048,D=1024,H=16,dh=65) on 8 TRN2 NeuronCores.

Sharding: batch x head-half. Core c handles batch c//2 and heads
(c%2)*8..(c%2)*8+8 (P-slice of 520). Each core computes its QKV projections,
attention, and a partial out-projection; the host sums the two partials per
batch and adds bo.

v2: single fused pipeline. k/v projection chains are interleaved with the
first two heads' attention rounds of q-block 0; q-chains / normalization /
out-projection are pumped between rounds of later heads so the PE never
drains. Rounds are software-pipelined (scores of round r+1 are enqueued
before the AV matmuls of round r, so the PE does not sit behind exp/mask).

Bias handling (no bias matmuls at all):
  - scores((q+bq)(k+bk)) = q~k~ + bq.k~ + [q~.bk + bq.bk]; the bracketed
    terms are constant along the softmax axis (keys) and cancel in the
    normalized softmax, so they are dropped. bq.k~ is folded in as a 66th
    contraction row: the k-projection weight gains one output column per
    head (Wk_h^T bq_h) and the q side carries a constant-1 row.
  - bv enters the output as (bv @ Wo^T), a constant row folded into bo on
    the host.

Softmax runs unnormalized (score magnitudes are small); the row-sum is
harvested from a trailing ones-column in V and divided out at the end.
Compute dtype bf16 (fp32 PSUM accumulation).
"""

import math
import sys
from contextlib import ExitStack

import numpy as np
import ml_dtypes

sys.path.insert(0, "/opt/trn_rl_repo")

import concourse.bass as bass
import concourse.mybir as mybir
import concourse.tile as tile_mod
from concourse.bass_utils import run_bass_kernel_spmd
from concourse.masks import make_identity
from concourse.vector_clock import ScopedClock

# ---------------------------------------------------------------------------
# Patch for this container's walrus build: it rejects instructions carrying
# more than one semaphore wait ("Too many sync wait commands"), but Tile's
# wait assigner freely attaches several. Split excess waits onto bass_nofuse
# InstNoOp carriers on the same engine, committed immediately before the
# instruction (same-engine program order => over-synchronization only).
# ---------------------------------------------------------------------------
_MAX_WAITS = 1

_orig_commit = tile_mod.TileContext._commit_instruction


def _split_waits(self, inst, commit):
    si = inst.sync_info
    if si is None or len(si.on_wait) <= _MAX_WAITS:
        return
    waits = list(si.on_wait)
    sem_w = [w for w in waits if getattr(w, "sync_type", "semaphore") == "semaphore"]
    other_w = [w for w in waits if getattr(w, "sync_type", "semaphore") != "semaphore"]
    keep_budget = _MAX_WAITS - len(other_w)
    if keep_budget < 0:
        return
    keep = other_w + (sem_w[-keep_budget:] if keep_budget > 0 else [])
    excess = sem_w[: len(sem_w) - max(keep_budget, 0)]
    if not excess:
        return
    for i, w in enumerate(excess):
        nop = mybir.InstNoOp(
            name=f"{inst.name}-sw{i}",
            sync_info=mybir.SyncInfo(on_wait=[w], on_update=[]),
            bass_nofuse=True,
            engine=inst.engine,
        )
        commit(nop)
    inst.sync_info = mybir.SyncInfo(on_wait=keep, on_update=list(si.on_update))


def _patched_commit(self, inst, lazy_reg_writes: bool = True):
    if inst.engine != mybir.EngineType.Unassigned:
        _split_waits(self, inst, lambda n: _orig_commit(self, n, False))
    return _orig_commit(self, inst, lazy_reg_writes)


def _patched_drain_and_barrier(self, tick_clock, wait_clock):
    drain_inst = self.nc.sync.drain()
    wait_clock.add_sem_waits(
        drain_inst.ins, ScopedClock({None: tick_clock.global_clock})
    )
    si = drain_inst.ins.sync_info
    if si is not None and len(si.on_wait) > _MAX_WAITS:
        waits = list(si.on_wait)
        drain_inst.ins.sync_info = mybir.SyncInfo(
            on_wait=waits[:_MAX_WAITS], on_update=list(si.on_update)
        )
        for w in waits[_MAX_WAITS:]:
            n = self.nc.sync.nop(nofuse=True)
            n.ins.sync_info = mybir.SyncInfo(on_wait=[w], on_update=[])
    self.nc.all_engine_barrier()
    popped = self.nc._tile_sem_poison_stack.pop()
    assert popped is self._sem_poison
    self.nc.clear_and_free_semaphores(list(self.sems.allocated().values()))
    self.nc.all_engine_barrier()


tile_mod.TileContext._commit_instruction = _patched_commit
tile_mod.TileContext._drain_and_barrier = _patched_drain_and_barrier

# ---------------------------------------------------------------------------

B, S, D, H = 4, 2048, 1024, 16
DH = D // H + 1          # 65
DHE = DH + 1             # 66: +1 contraction row carrying bq.k~
P = H * DH               # 1040
HPC = H // 2             # 8 heads per core
PC = HPC * DH            # 520, per-core P slice
PCE = HPC * DHE          # 528, k-weight slice incl. bias-fold column
N_CORES = 8

MT = S // 128            # 16 row blocks / k tiles
KT = 16                  # k tiles per attention row
QB = 4                   # q blocks of 512
QW = 512
RKT = 2                  # k-tiles per score round
NR = KT // RKT           # 8 rounds per head

F32 = mybir.dt.float32
BF16 = mybir.dt.bfloat16
BF = ml_dtypes.bfloat16

# packed 128-row k-tile ranges of the 520-row concatT / WoT
PKT = [(0, 128), (128, 256), (256, 384), (384, 512), (512, 520)]

_BUILT = {}


def _build_nc():
    nc = bass.Bass("TRN2", target_bir_lowering=False, debug=False,
                   num_devices=N_CORES)

    xq_d = nc.dram_tensor("xq", [D, S], BF16, kind="ExternalInput").ap()
    xk_d = nc.dram_tensor("xk", [D, S], BF16, kind="ExternalInput").ap()
    xv_d = nc.dram_tensor("xv", [D, S], BF16, kind="ExternalInput").ap()
    # maskH[qb, p, j*QW+q] = maskT[j*128+p, qb*512+q] (multiplicative 0/1)
    mh = nc.dram_tensor("maskH", [QB, 128, KT * QW], BF16,
                        kind="ExternalInput").ap()
    wq_d = nc.dram_tensor("wqT", [D, PC], BF16, kind="ExternalInput").ap()
    wk_d = nc.dram_tensor("wkT", [D, PCE], BF16, kind="ExternalInput").ap()
    wv_d = nc.dram_tensor("wvT", [D, PC], BF16, kind="ExternalInput").ap()
    wo_d = nc.dram_tensor("woT", [PC, D], BF16, kind="ExternalInput").ap()
    sel8_d = nc.dram_tensor("sel8", [HPC, PC], BF16,
                            kind="ExternalInput").ap()
    out = nc.dram_tensor("out", [S, D], F32, kind="ExternalOutput").ap()

    inv_sqrt = 1.0 / math.sqrt(float(DH))

    with tile_mod.TileContext(nc) as tc:
        stk = ExitStack()
        pconst = stk.enter_context(tc.tile_pool(name="const", bufs=1))
        pkT = stk.enter_context(tc.tile_pool(name="pkT", bufs=1))
        pqT = stk.enter_context(tc.tile_pool(name="pqT", bufs=1))
        pvh = stk.enter_context(tc.tile_pool(name="pvh", bufs=1))
        prs = stk.enter_context(tc.tile_pool(name="prs", bufs=2))
        puov = stk.enter_context(tc.tile_pool(name="puov", bufs=10))
        ppt = stk.enter_context(tc.tile_pool(name="ppt", bufs=4))
        pmask = stk.enter_context(tc.tile_pool(name="pmask", bufs=3))
        prow = stk.enter_context(tc.tile_pool(name="prow", bufs=2))
        pwq = stk.enter_context(tc.tile_pool(name="pwq", bufs=1))
        pxq = stk.enter_context(tc.tile_pool(name="pxq", bufs=8))
        psS = stk.enter_context(tc.tile_pool(name="psS", bufs=2, space="PSUM"))
        psV = stk.enter_context(tc.tile_pool(name="psV", bufs=2, space="PSUM"))
        psA = stk.enter_context(tc.tile_pool(name="psA", bufs=2, space="PSUM"))

        # ---- persistent SBUF state ----------------------------------------
        ident = pconst.tile([128, 128], BF16, tag="ident")
        make_identity(nc, ident[:])

        kT = pkT.tile([DHE, HPC, S], BF16, tag="kT")
        qTs = [pqT.tile([DHE, HPC, QW], BF16, tag=f"qT{i}", name=f"qT{i}")
               for i in range(2)]
        for qt in qTs:
            # const-1 row at partition 65 (bias fold). Engine ops need a
            # 32-aligned partition base, so also touch row 64 -- every
            # q-chain overwrites rows 0..64 afterwards.
            nc.gpsimd.memset(qt[64:DHE, :, :], 1.0)
        vh = [pvh.tile([128, HPC, DH + 1], BF16, tag=f"vh{j}", name=f"vh{j}")
              for j in range(MT)]
        for j in range(MT):
            nc.gpsimd.memset(vh[j][:, :, DH:DH + 1], 1.0)  # row-sum column

        rsall = [prs.tile([HPC, QW], BF16, tag=f"rsall{i}", bufs=1,
                          name=f"rsall{i}") for i in range(2)]
        rcall = [prs.tile([HPC, QW], BF16, tag=f"rcall{i}", bufs=1,
                          name=f"rcall{i}") for i in range(2)]
        # split row-sum tiles for the last q-block: lets the tail overlap
        # normalization of heads 0-3 with heads 4-7's attention
        ones8 = prs.tile([HPC, QW], F32, tag="ones8", bufs=1, name="ones8")
        nc.gpsimd.memset(ones8[:], 1.0)
        rsA3 = prs.tile([4, QW], BF16, tag="rsA3", bufs=1, name="rsA3")
        rsB3 = prs.tile([4, QW], BF16, tag="rsB3", bufs=1, name="rsB3")
        rcA3 = prs.tile([4, QW], BF16, tag="rcA3", bufs=1, name="rcA3")
        rcB3 = prs.tile([4, QW], BF16, tag="rcB3", bufs=1, name="rcB3")

        # ---- phase-0 weight / x pools (freed after qb0) --------------------
        kv_stk = ExitStack()
        pwkv = kv_stk.enter_context(tc.tile_pool(name="pwkv", bufs=1))
        pxk = kv_stk.enter_context(tc.tile_pool(name="pxk", bufs=12))
        pxv = kv_stk.enter_context(tc.tile_pool(name="pxv", bufs=12))

        uov_map = {}
        mask_tiles = {}
        xmap = {}

        def load_w(pool, dram, tag, de, eng=None):
            ts = []
            for d in range(8):
                t = pool.tile([128, HPC, de], BF16, tag=f"{tag}{d}",
                              bufs=1, name=f"{tag}{d}")
                (eng or nc.sync).dma_start(t[:], dram[d * 128:(d + 1) * 128, :])
                ts.append(t)
            return ts

        def load_x_chunk(which, pool, dram, c, tag, eng=None):
            ts = []
            for d in range(8):
                t = pool.tile([128, QW], BF16, tag=tag,
                              name=f"x{which}{c}_{d}")
                (eng or nc.sync).dma_start(
                    t[:], dram[d * 128:(d + 1) * 128, c * QW:(c + 1) * QW])
                ts.append(t)
            xmap[(which, c)] = ts

        def load_mask(qb):
            mts = []
            for hf in range(2):
                mt = pmask.tile([128, KT // 2, QW], BF16, tag="mask",
                                name=f"mask{qb}_{hf}")
                nc.sync.dma_start(
                    mt[:], mh[qb, :, hf * (KT // 2) * QW:
                              (hf + 1) * (KT // 2) * QW])
                mts.append(mt)
            mask_tiles[qb] = mts

        # ---- projection chains ---------------------------------------------
        def half_chain(which, m, half, wts, xts, de, qb=None):
            hs = half * 4
            co = (m % 4) * 128
            ps = psA.tile([128, 4, de], F32, tag="psA",
                          name=f"{which}ps{m}_{half}")
            for d in range(8):
                nc.tensor.matmul(ps[:], xts[d][:, co:co + 128],
                                 wts[d][:, hs:hs + 4, :],
                                 start=(d == 0), stop=(d == 7))
            if which == "v":
                nc.vector.tensor_copy(vh[m][:, hs:hs + 4, 0:DH], ps[:])
                return
            row = prow.tile([128, 4, DHE], BF16, tag=f"row{which}",
                            name=f"{which}row{m}_{half}")
            nc.vector.tensor_copy(row[:, :, 0:de], ps[:])
            pstr = psA.tile([128, 4, 128], BF16, tag="psA",
                            name=f"{which}pstr{m}_{half}")
            for t in range(4):
                nc.tensor.transpose(pstr[0:de, t, :], row[:, t, 0:de],
                                    ident[:])
            if which == "k":
                nc.vector.tensor_copy(
                    kT[0:de, hs:hs + 4, m * 128:(m + 1) * 128],
                    pstr[0:de, :, :])
            else:
                nc.scalar.copy(
                    qTs[qb % 2][0:de, hs:hs + 4, co:co + 128],
                    pstr[0:de, :, :])

        def kchain(m):
            for half in range(2):
                half_chain("k", m, half, wk_t, xmap[("k", m // 4)], DHE)

        def vchain(m):
            for half in range(2):
                half_chain("v", m, half, wv_t, xmap[("v", m // 4)], DH)

        def qchain(qb, m):
            for half in range(2):
                half_chain("q", m, half, wq_t, xmap[("q", qb)], DH, qb=qb)

        # ---- attention ------------------------------------------------------
        def mk_uov(qb, h, ov):
            uov = puov.tile([DH + 1, QW], BF16, tag="uov",
                            name=f"uov{qb}_{h}")
            nc.vector.tensor_copy(uov[:], ov[:])
            uov_map[(qb, h)] = uov
            if qb == QB - 1:
                # split (A/B) row-sum gathers so the tail recip/norm of
                # heads 0-3 can overlap heads 4-7's attention
                dst = rsA3 if h < 4 else rsB3
                nc.gpsimd.dma_start(dst[h % 4:h % 4 + 1, :],
                                    uov[DH:DH + 1, :])
            else:
                nc.gpsimd.dma_start(rsall[qb % 2][h:h + 1, :],
                                    uov[DH:DH + 1, :])

        # phase-1 (qb0 heads 0/1): single-round granularity, paced by the
        # k/v chains between rounds, AV lag of 1 round.
        def attn_round(qb, h, r):
            qt = qTs[qb % 2]
            ss = psS.tile([128, RKT, QW], F32, tag="psS",
                          name=f"ss{qb}_{h}_{r}")
            for jj in range(RKT):
                j = r * RKT + jj
                nc.tensor.matmul(ss[:, jj, :],
                                 kT[0:DHE, h, j * 128:(j + 1) * 128],
                                 qt[0:DHE, h, :], start=True, stop=True)
            pt = ppt.tile([128, RKT, QW], BF16, tag="pt1",
                          name=f"p1_{qb}_{h}_{r}", bufs=3)
            nc.scalar.activation(pt[:], ss[:],
                                 mybir.ActivationFunctionType.Exp,
                                 scale=inv_sqrt)
            mt = mask_tiles[qb][r // (NR // 2)]
            rr = r % (NR // 2)
            nc.vector.tensor_mul(pt[:], pt[:],
                                 mt[:, rr * RKT:(rr + 1) * RKT, :])
            return pt

        def emit_av(h, r, pt, ov):
            for jj in range(RKT):
                j = r * RKT + jj
                nc.tensor.matmul(ov[:], vh[j][:, h, :], pt[:, jj, :],
                                 start=(j == 0), stop=(j == KT - 1))

        # steady state: one continuous stream of (head, round-pair) units
        # across qb boundaries. exp per round (PSUM-bound), mask-mul per
        # round-PAIR (halves DVE op count), AV pairs lag 2 units so the
        # scores->exp->mask latency (~5us) never stalls the PE FIFO.
        pend = []

        def flush_one():
            qb_, h_, g_, ptS, ov = pend.pop(0)
            for jj4 in range(2 * RKT):
                j = g_ * 2 * RKT + jj4
                nc.tensor.matmul(ov[:], vh[j][:, h_, :], ptS[:, jj4, :],
                                 start=(j == 0), stop=(j == KT - 1))
            if g_ == NR // 2 - 1:
                mk_uov(qb_, h_, ov)

        def pair_unit(qb, h, g, ov):
            qt = qTs[qb % 2]
            ptS = ppt.tile([128, 2 * RKT, QW], BF16, tag="pt",
                           name=f"pt{qb}_{h}_{g}")
            for rr in range(2):
                r = 2 * g + rr
                ss = psS.tile([128, RKT, QW], F32, tag="psS",
                              name=f"ss{qb}_{h}_{r}")
                for jj in range(RKT):
                    j = r * RKT + jj
                    nc.tensor.matmul(ss[:, jj, :],
                                     kT[0:DHE, h, j * 128:(j + 1) * 128],
                                     qt[0:DHE, h, :], start=True, stop=True)
                nc.scalar.activation(ptS[:, rr * RKT:(rr + 1) * RKT, :],
                                     ss[:],
                                     mybir.ActivationFunctionType.Exp,
                                     scale=inv_sqrt)
            mt = mask_tiles[qb][g // 2]
            gg = g % 2
            nc.vector.tensor_mul(ptS[:], ptS[:],
                                 mt[:, gg * 2 * RKT:(gg + 1) * 2 * RKT, :])
            pend.append((qb, h, g, ptS, ov))
            if len(pend) > 2:
                flush_one()

        def attn_head(qb, h, items):
            ov = psV.tile([DH + 1, QW], F32, tag="ov", name=f"ov{qb}_{h}")
            for g in range(NR // 2):
                pair_unit(qb, h, g, ov)
                if g in (1, 2) and items:
                    items.pop(0)()
            while items:
                items.pop(0)()

        # ---- normalization + out-projection ---------------------------------
        def recip_t(src, dst, nm, eng=None):
            e = eng or nc.vector
            p = src.shape[0]
            rsf = prs.tile([p, QW], F32, tag=f"rsf{p}", name=f"rsf{nm}")
            e.tensor_copy(rsf[:], src[:])
            rcf = prs.tile([p, QW], F32, tag=f"rcf{p}", name=f"rcf{nm}")
            e.reciprocal(rcf[:], rsf[:])
            e.tensor_copy(dst[:], rcf[:])

        def recip_items(src, dst, nm):
            # the iterative reciprocal occupies the DVE for ~4us; split it
            # into three pump items so mask-multiplies interleave and the
            # exp stream never starves
            p = src.shape[0]
            rsf = prs.tile([p, QW], F32, tag=f"rsf{p}", name=f"rsf{nm}")
            rcf = prs.tile([p, QW], F32, tag=f"rcf{p}", name=f"rcf{nm}")

            def p1():
                nc.vector.tensor_copy(rsf[:], src[:])
                nc.vector.reciprocal(rcf[:, 0:QW // 2], rsf[:, 0:QW // 2])

            def p2():
                nc.vector.reciprocal(rcf[:, QW // 2:], rsf[:, QW // 2:])

            def p3():
                nc.vector.tensor_copy(dst[:], rcf[:])

            return [p1, p2, p3]

        def recip(qb):
            recip_t(rsall[qb % 2], rcall[qb % 2], f"u{qb}")

        def norm1(qb, h):
            rbp = psA.tile([128, QW], F32, tag="psA", name=f"rbp{qb}_{h}")
            if qb == QB - 1:
                sel, rc = (selA_t, rcA3) if h < 4 else (selB_t, rcB3)
                nc.tensor.matmul(rbp[0:DH, :], sel[:, h * DH:(h + 1) * DH],
                                 rc[:], start=True, stop=True)
            else:
                nc.tensor.matmul(rbp[0:DH, :],
                                 sel8_t[:, h * DH:(h + 1) * DH],
                                 rcall[qb % 2][:], start=True, stop=True)
            cch = pcch.tile([DH, QW], BF16, tag="cch", name=f"cch{qb}_{h}")
            nc.vector.tensor_mul(cch[:], rbp[0:DH, :],
                                 uov_map[(qb, h)][0:DH, :])
            r0 = h * DH
            for i, (a, b) in enumerate(PKT):
                lo, hi = max(r0, a), min(r0 + DH, b)
                if lo < hi:
                    # sync engine: keeps these off the gpsimd clock that
                    # the out-proj LDWEIGHTS conservatively waits on
                    nc.sync.dma_start(
                        ccp[i][lo - a:hi - a, qb * QW:(qb + 1) * QW],
                        cch[lo - r0:hi - r0, :])

        def outproj_half(gm, n):
            ps = psA.tile([128, QW], F32, tag="psA", name=f"pop{gm}_{n}")
            for i, (a, b) in enumerate(PKT):
                nc.tensor.matmul(ps[:], ccp[i][:, gm * 128:(gm + 1) * 128],
                                 wop[i][:, n * QW:(n + 1) * QW],
                                 start=(i == 0), stop=(i == len(PKT) - 1))
            osb = posb.tile([128, QW], F32, tag="osb", name=f"osb{gm}_{n}")
            nc.vector.tensor_copy(osb[:], ps[:])
            nc.gpsimd.dma_start(
                out[gm * 128:(gm + 1) * 128, n * QW:(n + 1) * QW], osb[:])

        # =====================================================================
        # prologue: weight + first-chunk loads, q chunk-0 chains
        # =====================================================================
        # wq on sync, xq on gpsimd: the two DMA queues run in parallel so
        # the first q-chain starts ~7us in instead of ~16us
        wq_t = load_w(pwq, wq_d, "wq", DH)
        load_x_chunk("q", pxq, xq_d, 0, "xq", eng=nc.gpsimd)
        wk_t = load_w(pwkv, wk_d, "wk", DHE)
        load_x_chunk("k", pxk, xk_d, 0, "xk")
        wv_t = load_w(pwkv, wv_d, "wv", DH, eng=nc.gpsimd)
        load_x_chunk("v", pxv, xv_d, 0, "xv", eng=nc.gpsimd)
        load_mask(0)

        for m in range(4):
            qchain(0, m)

        # =====================================================================
        # qb0: k/v chains paced against heads 0,1; then heads 2..7
        # =====================================================================
        ov01 = [psV.tile([DH + 1, QW], F32, tag="ov", name=f"ov0_{h}")
                for h in range(2)]
        prev01 = [None, None]
        for r in range(NR):
            for m in (2 * r, 2 * r + 1):
                kchain(m)
                vchain(m)
                if m % 4 == 1 and m // 4 < 3:
                    load_x_chunk("k", pxk, xk_d, m // 4 + 1, "xk")
                    load_x_chunk("v", pxv, xv_d, m // 4 + 1, "xv")
            for h in range(2):
                pt = attn_round(0, h, r)
                if prev01[h] is not None:
                    emit_av(h, r - 1, prev01[h], ov01[h])
                prev01[h] = pt
        for h in range(2):
            emit_av(h, NR - 1, prev01[h], ov01[h])
            mk_uov(0, h, ov01[h])

        for h in range(2, HPC):
            items = []
            if h == 2:
                def _ld1():
                    load_x_chunk("q", pxq, xq_d, 1, "xq")
                    load_mask(1)
                items.append(_ld1)
            if 3 <= h <= 6:
                items.append(lambda m=h - 3: qchain(1, m))
            attn_head(0, h, items)

        kv_stk.close()   # free wk/wv/xk/xv SBUF before phase 2

        # =====================================================================
        # qb 1..3 with interleaved norm/outproj of qb-1; then epilogue
        # =====================================================================
        with tc.tile_pool(name="pccp", bufs=1) as pccp, \
             tc.tile_pool(name="pwo", bufs=1) as pwo, \
             tc.tile_pool(name="pcch", bufs=4) as pcch, \
             tc.tile_pool(name="posb", bufs=2) as posb:

            ccp = [pccp.tile([b - a, S], BF16, tag=f"ccp{i}", name=f"ccp{i}")
                   for i, (a, b) in enumerate(PKT)]
            wop = []
            for i, (a, b) in enumerate(PKT):
                w = pwo.tile([b - a, D], BF16, tag=f"wop{i}", name=f"wop{i}")
                nc.sync.dma_start(w[:], wo_d[a:b, :])
                wop.append(w)
            sel8_t = pwo.tile([HPC, PC], BF16, tag="sel8")
            nc.sync.dma_start(sel8_t[:], sel8_d[:])
            selA_t = pwo.tile([4, PC], BF16, tag="selA")
            nc.sync.dma_start(selA_t[:], sel8_d[0:4, :])
            selB_t = pwo.tile([4, PC], BF16, tag="selB")
            nc.sync.dma_start(selB_t[:], sel8_d[4:HPC, :])

            def mk_items(qb, h):
                # work for qb-1 (norm+outproj), prefetch for qb+1, and for
                # qb3 the early normalization of its own heads 0-3.
                it = []
                if h == 0:
                    it += recip_items(rsall[(qb - 1) % 2],
                                      rcall[(qb - 1) % 2], f"u{qb - 1}")
                elif h <= 4:
                    it.append(lambda q=qb - 1, hh=2 * (h - 1): norm1(q, hh))
                    it.append(
                        lambda q=qb - 1, hh=2 * (h - 1) + 1: norm1(q, hh))
                    if qb >= 2 and h <= 2:   # leftover block of qb-2
                        gm = (qb - 2) * 4 + 3
                        it.append(lambda g=gm, n=h - 1: outproj_half(g, n))
                elif qb < 3 or h == 5:
                    gm = (qb - 1) * 4 + (h - 5)
                    it.append(lambda g=gm: outproj_half(g, 0))
                    it.append(lambda g=gm: outproj_half(g, 1))
                if qb < 3:
                    if h == 2:
                        def _ld(c=qb + 1):
                            load_x_chunk("q", pxq, xq_d, c, "xq")
                            load_mask(c)
                        it.append(_ld)
                    if 3 <= h <= 6:
                        it.append(lambda c=qb + 1, m=h - 3: qchain(c, m))
                else:
                    # qb2's outproj blocks 1..3 are deferred to the tail,
                    # where they keep the PE fed during the final row-sum
                    # gather + reciprocal; qb3's own heads 0-3 normalize here.
                    if h == 5:
                        rps = recip_items(rsA3, rcA3, "A3")
                        it.insert(0, rps[0])
                        it.insert(2, rps[1])
                        it.append(rps[2])
                    elif h == 6:
                        it.append(lambda: norm1(3, 0))
                        it.append(lambda: norm1(3, 1))
                    elif h == 7:
                        it.append(lambda: norm1(3, 2))
                        it.append(lambda: norm1(3, 3))
                return it

            for qb in range(1, QB):
                for h in range(HPC):
                    attn_head(qb, h, mk_items(qb, h))
            while pend:
                flush_one()

            # tail: the deferred qb2 out-proj blocks overlap the final
            # row-sum gather + reciprocal (emit them FIRST -- pool-slot
            # reuse waits on the issuing engine's full clock at emission,
            # so anything emitted after the recip would serialize on it);
            # then heads 4-7 of qb3 normalize and the last q-block
            # projects out.
            for gm in (9, 10, 11):
                outproj_half(gm, 0)
                outproj_half(gm, 1)
            recip_t(rsB3, rcB3, "B3", eng=nc.vector)
            # dummy matmuls bridge the PE-idle window while the reciprocal
            # chain runs, so HAM stays at full clock for the final out-proj
            for i in range(6):
                dss = psS.tile([128, RKT, QW], F32, tag="psS",
                               name=f"warm{i}")
                for jj in range(RKT):
                    nc.tensor.matmul(dss[:, jj, :], kT[0:DHE, 0, 0:128],
                                     qTs[1][0:DHE, 0, :],
                                     start=True, stop=True)
            for h in range(4, HPC):
                norm1(QB - 1, h)
            for gm in range((QB - 1) * 4, QB * 4):
                outproj_half(gm, 0)
                outproj_half(gm, 1)
        stk.close()

    return nc


def _prep_inputs(q, k, v, mask, Wq, bqv, Wk, bkv, Wv, bvv, Wo):
    """Per-core input maps (numpy, host-side shard + cast)."""
    in_maps = []
    sel8 = np.zeros((HPC, PC), np.float32)
    for h in range(HPC):
        sel8[h, h * DH:(h + 1) * DH] = 1.0
    sel8 = sel8.astype(BF)
    mask_h = {}
    for b in range(B):
        mt = (mask[b, 0] != 0).astype(np.float32).T  # [k, q]
        m4 = mt.reshape(KT, 128, QB, QW).transpose(2, 1, 0, 3)
        mask_h[b] = np.ascontiguousarray(m4.reshape(QB, 128, KT * QW)).astype(BF)
    for c in range(N_CORES):
        b, hh = c // 2, c % 2
        sl = slice(hh * PC, (hh + 1) * PC)
        Wk_l = Wk[sl, :]
        bq_l = bqv[sl]
        # k weights with one extra output column per head: Wk_h^T @ bq_h
        wkT_e = np.zeros((D, PCE), np.float32)
        for h in range(HPC):
            Wk_h = Wk_l[h * DH:(h + 1) * DH, :]
            wkT_e[:, h * DHE:h * DHE + DH] = Wk_h.T
            wkT_e[:, h * DHE + DH] = Wk_h.T @ bq_l[h * DH:(h + 1) * DH]
        in_maps.append({
            "xq": np.ascontiguousarray(q[b].T).astype(BF),
            "xk": np.ascontiguousarray(k[b].T).astype(BF),
            "xv": np.ascontiguousarray(v[b].T).astype(BF),
            "maskH": mask_h[b],
            "wqT": np.ascontiguousarray(Wq[sl, :].T).astype(BF),
            "wkT": wkT_e.astype(BF),
            "wvT": np.ascontiguousarray(Wv[sl, :].T).astype(BF),
            "woT": np.ascontiguousarray(Wo[:, sl].T).astype(BF),
            "sel8": sel8,
        })
    return in_maps


def run_sharded(in_maps, **kwargs):
    if "nc" not in _BUILT:
        _BUILT["nc"] = _build_nc()
    return run_bass_kernel_spmd(_BUILT["nc"], in_maps,
                                core_ids=list(range(N_CORES)), **kwargs)


def kernel(q, k, v, mask, Wq, bq, Wk, bk, Wv, bv, Wo, bo):
    q = np.asarray(q, np.float32)
    k = np.asarray(k, np.float32)
    v = np.asarray(v, np.float32)
    mask = np.asarray(mask)
    Wo32 = np.asarray(Wo, np.float32)
    bv32 = np.asarray(bv, np.float32)
    in_maps = _prep_inputs(q, k, v, mask,
                           np.asarray(Wq, np.float32), np.asarray(bq, np.float32),
                           np.asarray(Wk, np.float32), np.asarray(bk, np.float32),
                           np.asarray(Wv, np.float32), bv32, Wo32)
    res = run_sharded(in_maps)
    bo_eff = np.asarray(bo, np.float32) + bv32 @ Wo32.T
    out = np.empty((B, S, D), np.float32)
    for b in range(B):
        out[b] = res.results[2 * b]["out"] + res.results[2 * b + 1]["out"] + bo_eff
    return out


# revision 37
# speedup vs baseline: 1.0729x; 1.0729x over previous
"""Multi-head attention (B=4,S=2048,D=1024,H=16,dh=65) on 8 TRN2 NeuronCores.

Sharding: batch x head-half. Core c handles batch c//2 and heads
(c%2)*8..(c%2)*8+8 (P-slice of 520). Each core computes its QKV projections,
attention, and a partial out-projection; the host sums the two partials per
batch and adds bo.

v2: single fused pipeline. k/v projection chains are interleaved with the
first two heads' attention rounds of q-block 0; q-chains / normalization /
out-projection are pumped between rounds of later heads so the PE never
drains. Rounds are software-pipelined (scores of round r+1 are enqueued
before the AV matmuls of round r, so the PE does not sit behind exp/mask).

Bias handling (no bias matmuls at all):
  - scores((q+bq)(k+bk)) = q~k~ + bq.k~ + [q~.bk + bq.bk]; the bracketed
    terms are constant along the softmax axis (keys) and cancel in the
    normalized softmax, so they are dropped. bq.k~ is folded in as a 66th
    contraction row: the k-projection weight gains one output column per
    head (Wk_h^T bq_h) and the q side carries a constant-1 row.
  - bv enters the output as (bv @ Wo^T), a constant row folded into bo on
    the host.

Softmax runs unnormalized (score magnitudes are small); the row-sum is
harvested from a trailing ones-column in V and divided out at the end.
Compute dtype bf16 (fp32 PSUM accumulation).
"""

import math
import sys
from contextlib import ExitStack

import numpy as np
import ml_dtypes

sys.path.insert(0, "/opt/trn_rl_repo")

import concourse.bass as bass
import concourse.mybir as mybir
import concourse.tile as tile_mod
from concourse.bass_utils import run_bass_kernel_spmd
from concourse.masks import make_identity
from concourse.vector_clock import ScopedClock

# ---------------------------------------------------------------------------
# Patch for this container's walrus build: it rejects instructions carrying
# more than one semaphore wait ("Too many sync wait commands"), but Tile's
# wait assigner freely attaches several. Split excess waits onto bass_nofuse
# InstNoOp carriers on the same engine, committed immediately before the
# instruction (same-engine program order => over-synchronization only).
# ---------------------------------------------------------------------------
_MAX_WAITS = 1

_orig_commit = tile_mod.TileContext._commit_instruction


def _split_waits(self, inst, commit):
    si = inst.sync_info
    if si is None or len(si.on_wait) <= _MAX_WAITS:
        return
    waits = list(si.on_wait)
    sem_w = [w for w in waits if getattr(w, "sync_type", "semaphore") == "semaphore"]
    other_w = [w for w in waits if getattr(w, "sync_type", "semaphore") != "semaphore"]
    keep_budget = _MAX_WAITS - len(other_w)
    if keep_budget < 0:
        return
    keep = other_w + (sem_w[-keep_budget:] if keep_budget > 0 else [])
    excess = sem_w[: len(sem_w) - max(keep_budget, 0)]
    if not excess:
        return
    for i, w in enumerate(excess):
        nop = mybir.InstNoOp(
            name=f"{inst.name}-sw{i}",
            sync_info=mybir.SyncInfo(on_wait=[w], on_update=[]),
            bass_nofuse=True,
            engine=inst.engine,
        )
        commit(nop)
    inst.sync_info = mybir.SyncInfo(on_wait=keep, on_update=list(si.on_update))


def _patched_commit(self, inst, lazy_reg_writes: bool = True):
    if inst.engine != mybir.EngineType.Unassigned:
        _split_waits(self, inst, lambda n: _orig_commit(self, n, False))
    return _orig_commit(self, inst, lazy_reg_writes)


def _patched_drain_and_barrier(self, tick_clock, wait_clock):
    drain_inst = self.nc.sync.drain()
    wait_clock.add_sem_waits(
        drain_inst.ins, ScopedClock({None: tick_clock.global_clock})
    )
    si = drain_inst.ins.sync_info
    if si is not None and len(si.on_wait) > _MAX_WAITS:
        waits = list(si.on_wait)
        drain_inst.ins.sync_info = mybir.SyncInfo(
            on_wait=waits[:_MAX_WAITS], on_update=list(si.on_update)
        )
        for w in waits[_MAX_WAITS:]:
            n = self.nc.sync.nop(nofuse=True)
            n.ins.sync_info = mybir.SyncInfo(on_wait=[w], on_update=[])
    self.nc.all_engine_barrier()
    popped = self.nc._tile_sem_poison_stack.pop()
    assert popped is self._sem_poison
    self.nc.clear_and_free_semaphores(list(self.sems.allocated().values()))
    self.nc.all_engine_barrier()


tile_mod.TileContext._commit_instruction = _patched_commit
tile_mod.TileContext._drain_and_barrier = _patched_drain_and_barrier

# ---------------------------------------------------------------------------

B, S, D, H = 4, 2048, 1024, 16
DH = D // H + 1          # 65
DHE = DH + 1             # 66: +1 contraction row carrying bq.k~
P = H * DH               # 1040
HPC = H // 2             # 8 heads per core
PC = HPC * DH            # 520, per-core P slice
PCE = HPC * DHE          # 528, k-weight slice incl. bias-fold column
N_CORES = 8

MT = S // 128            # 16 row blocks / k tiles
KT = 16                  # k tiles per attention row
QB = 4                   # q blocks of 512
QW = 512
RKT = 2                  # k-tiles per score round
NR = KT // RKT           # 8 rounds per head

F32 = mybir.dt.float32
BF16 = mybir.dt.bfloat16
BF = ml_dtypes.bfloat16

# packed 128-row k-tile ranges of the 520-row concatT / WoT
PKT = [(0, 128), (128, 256), (256, 384), (384, 512), (512, 520)]

_BUILT = {}


def _build_nc():
    nc = bass.Bass("TRN2", target_bir_lowering=False, debug=False,
                   num_devices=N_CORES)

    xq_d = nc.dram_tensor("xq", [D, S], BF16, kind="ExternalInput").ap()
    xk_d = nc.dram_tensor("xk", [D, S], BF16, kind="ExternalInput").ap()
    xv_d = nc.dram_tensor("xv", [D, S], BF16, kind="ExternalInput").ap()
    # maskH[qb, p, j*QW+q] = maskT[j*128+p, qb*512+q] (multiplicative 0/1)
    mh = nc.dram_tensor("maskH", [QB, 128, KT * QW], BF16,
                        kind="ExternalInput").ap()
    wq_d = nc.dram_tensor("wqT", [D, PC], BF16, kind="ExternalInput").ap()
    wk_d = nc.dram_tensor("wkT", [D, PCE], BF16, kind="ExternalInput").ap()
    wv_d = nc.dram_tensor("wvT", [D, PC], BF16, kind="ExternalInput").ap()
    wo_d = nc.dram_tensor("woT", [PC, D], BF16, kind="ExternalInput").ap()
    sel8_d = nc.dram_tensor("sel8", [HPC, PC], BF16,
                            kind="ExternalInput").ap()
    out = nc.dram_tensor("out", [S, D], F32, kind="ExternalOutput").ap()

    inv_sqrt = 1.0 / math.sqrt(float(DH))

    with tile_mod.TileContext(nc) as tc:
        stk = ExitStack()
        pconst = stk.enter_context(tc.tile_pool(name="const", bufs=1))
        pkT = stk.enter_context(tc.tile_pool(name="pkT", bufs=1))
        pqT = stk.enter_context(tc.tile_pool(name="pqT", bufs=1))
        pvh = stk.enter_context(tc.tile_pool(name="pvh", bufs=1))
        prs = stk.enter_context(tc.tile_pool(name="prs", bufs=2))
        puov = stk.enter_context(tc.tile_pool(name="puov", bufs=10))
        ppt = stk.enter_context(tc.tile_pool(name="ppt", bufs=4))
        pmask = stk.enter_context(tc.tile_pool(name="pmask", bufs=3))
        prow = stk.enter_context(tc.tile_pool(name="prow", bufs=2))
        pwq = stk.enter_context(tc.tile_pool(name="pwq", bufs=1))
        pxq = stk.enter_context(tc.tile_pool(name="pxq", bufs=8))
        psS = stk.enter_context(tc.tile_pool(name="psS", bufs=2, space="PSUM"))
        psV = stk.enter_context(tc.tile_pool(name="psV", bufs=2, space="PSUM"))
        psA = stk.enter_context(tc.tile_pool(name="psA", bufs=2, space="PSUM"))

        # ---- persistent SBUF state ----------------------------------------
        ident = pconst.tile([128, 128], BF16, tag="ident")
        make_identity(nc, ident[:])

        kT = pkT.tile([DHE, HPC, S], BF16, tag="kT")
        qTs = [pqT.tile([DHE, HPC, QW], BF16, tag=f"qT{i}", name=f"qT{i}")
               for i in range(2)]
        for qt in qTs:
            # const-1 row at partition 65 (bias fold). Engine ops need a
            # 32-aligned partition base, so also touch row 64 -- every
            # q-chain overwrites rows 0..64 afterwards.
            nc.vector.memset(qt[64:DHE, :, :], 1.0)
        vh = [pvh.tile([128, HPC, DH + 1], BF16, tag=f"vh{j}", name=f"vh{j}")
              for j in range(MT)]
        for j in range(MT):
            nc.vector.memset(vh[j][:, :, DH:DH + 1], 1.0)  # row-sum column

        rsall = [prs.tile([HPC, QW], BF16, tag=f"rsall{i}", bufs=1,
                          name=f"rsall{i}") for i in range(2)]
        rcall = [prs.tile([HPC, QW], BF16, tag=f"rcall{i}", bufs=1,
                          name=f"rcall{i}") for i in range(2)]
        # split row-sum tiles for the last q-block: lets the tail overlap
        # normalization of heads 0-3 with heads 4-7's attention
        rsA3 = prs.tile([4, QW], BF16, tag="rsA3", bufs=1, name="rsA3")
        rsB3 = prs.tile([4, QW], BF16, tag="rsB3", bufs=1, name="rsB3")
        rcA3 = prs.tile([4, QW], BF16, tag="rcA3", bufs=1, name="rcA3")
        rcB3 = prs.tile([4, QW], BF16, tag="rcB3", bufs=1, name="rcB3")

        # ---- phase-0 weight / x pools (freed after qb0) --------------------
        kv_stk = ExitStack()
        pwkv = kv_stk.enter_context(tc.tile_pool(name="pwkv", bufs=1))
        pxk = kv_stk.enter_context(tc.tile_pool(name="pxk", bufs=12))
        pxv = kv_stk.enter_context(tc.tile_pool(name="pxv", bufs=12))

        uov_map = {}
        mask_tiles = {}
        xmap = {}

        def load_w(pool, dram, tag, de, eng=None):
            ts = []
            for d in range(8):
                t = pool.tile([128, HPC, de], BF16, tag=f"{tag}{d}",
                              bufs=1, name=f"{tag}{d}")
                (eng or nc.sync).dma_start(t[:], dram[d * 128:(d + 1) * 128, :])
                ts.append(t)
            return ts

        def load_x_chunk(which, pool, dram, c, tag, eng=None):
            ts = []
            for d in range(8):
                t = pool.tile([128, QW], BF16, tag=tag,
                              name=f"x{which}{c}_{d}")
                (eng or nc.sync).dma_start(
                    t[:], dram[d * 128:(d + 1) * 128, c * QW:(c + 1) * QW])
                ts.append(t)
            xmap[(which, c)] = ts

        def load_mask(qb):
            mts = []
            for hf in range(2):
                mt = pmask.tile([128, KT // 2, QW], BF16, tag="mask",
                                name=f"mask{qb}_{hf}")
                nc.sync.dma_start(
                    mt[:], mh[qb, :, hf * (KT // 2) * QW:
                              (hf + 1) * (KT // 2) * QW])
                mts.append(mt)
            mask_tiles[qb] = mts

        # ---- projection chains ---------------------------------------------
        def half_chain(which, m, half, wts, xts, de, qb=None):
            hs = half * 4
            co = (m % 4) * 128
            ps = psA.tile([128, 4, de], F32, tag="psA",
                          name=f"{which}ps{m}_{half}")
            for d in range(8):
                nc.tensor.matmul(ps[:], xts[d][:, co:co + 128],
                                 wts[d][:, hs:hs + 4, :],
                                 start=(d == 0), stop=(d == 7))
            if which == "v":
                nc.vector.tensor_copy(vh[m][:, hs:hs + 4, 0:DH], ps[:])
                return
            row = prow.tile([128, 4, DHE], BF16, tag=f"row{which}",
                            name=f"{which}row{m}_{half}")
            nc.vector.tensor_copy(row[:, :, 0:de], ps[:])
            pstr = psA.tile([128, 4, 128], BF16, tag="psA",
                            name=f"{which}pstr{m}_{half}")
            for t in range(4):
                nc.tensor.transpose(pstr[0:de, t, :], row[:, t, 0:de],
                                    ident[:])
            if which == "k":
                nc.vector.tensor_copy(
                    kT[0:de, hs:hs + 4, m * 128:(m + 1) * 128],
                    pstr[0:de, :, :])
            else:
                nc.scalar.copy(
                    qTs[qb % 2][0:de, hs:hs + 4, co:co + 128],
                    pstr[0:de, :, :])

        def kchain(m):
            for half in range(2):
                half_chain("k", m, half, wk_t, xmap[("k", m // 4)], DHE)

        def vchain(m):
            for half in range(2):
                half_chain("v", m, half, wv_t, xmap[("v", m // 4)], DH)

        def qchain(qb, m):
            for half in range(2):
                half_chain("q", m, half, wq_t, xmap[("q", qb)], DH, qb=qb)

        # ---- attention ------------------------------------------------------
        def mk_uov(qb, h, ov):
            uov = puov.tile([DH + 1, QW], BF16, tag="uov",
                            name=f"uov{qb}_{h}")
            nc.vector.tensor_copy(uov[:], ov[:])
            uov_map[(qb, h)] = uov
            if qb == QB - 1:
                # split (A/B) row-sum gathers so the tail recip/norm of
                # heads 0-3 can overlap heads 4-7's attention
                dst = rsA3 if h < 4 else rsB3
                nc.gpsimd.dma_start(dst[h % 4:h % 4 + 1, :],
                                    uov[DH:DH + 1, :])
            else:
                nc.gpsimd.dma_start(rsall[qb % 2][h:h + 1, :],
                                    uov[DH:DH + 1, :])

        # phase-1 (qb0 heads 0/1): single-round granularity, paced by the
        # k/v chains between rounds, AV lag of 1 round.
        def attn_round(qb, h, r):
            qt = qTs[qb % 2]
            ss = psS.tile([128, RKT, QW], F32, tag="psS",
                          name=f"ss{qb}_{h}_{r}")
            for jj in range(RKT):
                j = r * RKT + jj
                nc.tensor.matmul(ss[:, jj, :],
                                 kT[0:DHE, h, j * 128:(j + 1) * 128],
                                 qt[0:DHE, h, :], start=True, stop=True)
            pt = ppt.tile([128, RKT, QW], BF16, tag="pt1",
                          name=f"p1_{qb}_{h}_{r}", bufs=3)
            nc.scalar.activation(pt[:], ss[:],
                                 mybir.ActivationFunctionType.Exp,
                                 scale=inv_sqrt)
            mt = mask_tiles[qb][r // (NR // 2)]
            rr = r % (NR // 2)
            nc.vector.tensor_mul(pt[:], pt[:],
                                 mt[:, rr * RKT:(rr + 1) * RKT, :])
            return pt

        def emit_av(h, r, pt, ov):
            for jj in range(RKT):
                j = r * RKT + jj
                nc.tensor.matmul(ov[:], vh[j][:, h, :], pt[:, jj, :],
                                 start=(j == 0), stop=(j == KT - 1))

        # steady state: one continuous stream of (head, round-pair) units
        # across qb boundaries. exp per round (PSUM-bound), mask-mul per
        # round-PAIR (halves DVE op count), AV pairs lag 2 units so the
        # scores->exp->mask latency (~5us) never stalls the PE FIFO.
        pend = []

        def flush_one():
            qb_, h_, g_, ptS, ov = pend.pop(0)
            for jj4 in range(2 * RKT):
                j = g_ * 2 * RKT + jj4
                nc.tensor.matmul(ov[:], vh[j][:, h_, :], ptS[:, jj4, :],
                                 start=(j == 0), stop=(j == KT - 1))
            if g_ == NR // 2 - 1:
                mk_uov(qb_, h_, ov)

        def pair_unit(qb, h, g, ov):
            qt = qTs[qb % 2]
            ptS = ppt.tile([128, 2 * RKT, QW], BF16, tag="pt",
                           name=f"pt{qb}_{h}_{g}")
            for rr in range(2):
                r = 2 * g + rr
                ss = psS.tile([128, RKT, QW], F32, tag="psS",
                              name=f"ss{qb}_{h}_{r}")
                for jj in range(RKT):
                    j = r * RKT + jj
                    nc.tensor.matmul(ss[:, jj, :],
                                     kT[0:DHE, h, j * 128:(j + 1) * 128],
                                     qt[0:DHE, h, :], start=True, stop=True)
                nc.scalar.activation(ptS[:, rr * RKT:(rr + 1) * RKT, :],
                                     ss[:],
                                     mybir.ActivationFunctionType.Exp,
                                     scale=inv_sqrt)
            mt = mask_tiles[qb][g // 2]
            gg = g % 2
            nc.vector.tensor_mul(ptS[:], ptS[:],
                                 mt[:, gg * 2 * RKT:(gg + 1) * 2 * RKT, :])
            pend.append((qb, h, g, ptS, ov))
            if len(pend) > 2:
                flush_one()

        def attn_head(qb, h, items):
            ov = psV.tile([DH + 1, QW], F32, tag="ov", name=f"ov{qb}_{h}")
            for g in range(NR // 2):
                pair_unit(qb, h, g, ov)
                if g in (1, 2) and items:
                    items.pop(0)()
            while items:
                items.pop(0)()

        # ---- normalization + out-projection ---------------------------------
        def recip_t(src, dst, nm, eng=None):
            e = eng or nc.vector
            p = src.shape[0]
            rsf = prs.tile([p, QW], F32, tag=f"rsf{p}", name=f"rsf{nm}")
            e.tensor_copy(rsf[:], src[:])
            rcf = prs.tile([p, QW], F32, tag=f"rcf{p}", name=f"rcf{nm}")
            e.reciprocal(rcf[:], rsf[:])
            e.tensor_copy(dst[:], rcf[:])

        def recip_items(src, dst, nm):
            # the iterative reciprocal occupies the DVE for ~4us; split it
            # into three pump items so mask-multiplies interleave and the
            # exp stream never starves
            p = src.shape[0]
            rsf = prs.tile([p, QW], F32, tag=f"rsf{p}", name=f"rsf{nm}")
            rcf = prs.tile([p, QW], F32, tag=f"rcf{p}", name=f"rcf{nm}")

            def p1():
                nc.vector.tensor_copy(rsf[:], src[:])
                nc.vector.reciprocal(rcf[:, 0:QW // 2], rsf[:, 0:QW // 2])

            def p2():
                nc.vector.reciprocal(rcf[:, QW // 2:], rsf[:, QW // 2:])

            def p3():
                nc.vector.tensor_copy(dst[:], rcf[:])

            return [p1, p2, p3]

        def recip(qb):
            recip_t(rsall[qb % 2], rcall[qb % 2], f"u{qb}")

        def norm1(qb, h):
            rbp = psA.tile([128, QW], F32, tag="psA", name=f"rbp{qb}_{h}")
            if qb == QB - 1:
                sel, rc = (selA_t, rcA3) if h < 4 else (selB_t, rcB3)
                nc.tensor.matmul(rbp[0:DH, :], sel[:, h * DH:(h + 1) * DH],
                                 rc[:], start=True, stop=True)
            else:
                nc.tensor.matmul(rbp[0:DH, :],
                                 sel8_t[:, h * DH:(h + 1) * DH],
                                 rcall[qb % 2][:], start=True, stop=True)
            cch = pcch.tile([DH, QW], BF16, tag="cch", name=f"cch{qb}_{h}")
            nc.vector.tensor_mul(cch[:], rbp[0:DH, :],
                                 uov_map[(qb, h)][0:DH, :])
            r0 = h * DH
            for i, (a, b) in enumerate(PKT):
                lo, hi = max(r0, a), min(r0 + DH, b)
                if lo < hi:
                    # sync engine: keeps these off the gpsimd clock that
                    # the out-proj LDWEIGHTS conservatively waits on
                    nc.sync.dma_start(
                        ccp[i][lo - a:hi - a, qb * QW:(qb + 1) * QW],
                        cch[lo - r0:hi - r0, :])

        def outproj_half(gm, n):
            ps = psA.tile([128, QW], F32, tag="psA", name=f"pop{gm}_{n}")
            for i, (a, b) in enumerate(PKT):
                nc.tensor.matmul(ps[:], ccp[i][:, gm * 128:(gm + 1) * 128],
                                 wop[i][:, n * QW:(n + 1) * QW],
                                 start=(i == 0), stop=(i == len(PKT) - 1))
            osb = posb.tile([128, QW], F32, tag="osb", name=f"osb{gm}_{n}")
            nc.vector.tensor_copy(osb[:], ps[:])
            nc.gpsimd.dma_start(
                out[gm * 128:(gm + 1) * 128, n * QW:(n + 1) * QW], osb[:])

        # =====================================================================
        # prologue: weight + first-chunk loads, q chunk-0 chains
        # =====================================================================
        # wq on sync, xq on gpsimd: the two DMA queues run in parallel so
        # the first q-chain starts ~7us in instead of ~16us
        wq_t = load_w(pwq, wq_d, "wq", DH)
        load_x_chunk("q", pxq, xq_d, 0, "xq", eng=nc.gpsimd)
        wk_t = load_w(pwkv, wk_d, "wk", DHE)
        load_x_chunk("k", pxk, xk_d, 0, "xk")
        wv_t = load_w(pwkv, wv_d, "wv", DH, eng=nc.gpsimd)
        load_x_chunk("v", pxv, xv_d, 0, "xv", eng=nc.gpsimd)
        load_mask(0)

        for m in range(4):
            qchain(0, m)

        # =====================================================================
        # qb0: k/v chains paced against heads 0,1; then heads 2..7
        # =====================================================================
        ov01 = [psV.tile([DH + 1, QW], F32, tag="ov", name=f"ov0_{h}")
                for h in range(2)]
        prev01 = [None, None]
        for r in range(NR):
            for m in (2 * r, 2 * r + 1):
                kchain(m)
                vchain(m)
                if m % 4 == 1 and m // 4 < 3:
                    load_x_chunk("k", pxk, xk_d, m // 4 + 1, "xk")
                    load_x_chunk("v", pxv, xv_d, m // 4 + 1, "xv")
            for h in range(2):
                pt = attn_round(0, h, r)
                if prev01[h] is not None:
                    emit_av(h, r - 1, prev01[h], ov01[h])
                prev01[h] = pt
        for h in range(2):
            emit_av(h, NR - 1, prev01[h], ov01[h])
            mk_uov(0, h, ov01[h])

        for h in range(2, HPC):
            items = []
            if h == 2:
                def _ld1():
                    load_x_chunk("q", pxq, xq_d, 1, "xq")
                    load_mask(1)
                items.append(_ld1)
            if 3 <= h <= 6:
                items.append(lambda m=h - 3: qchain(1, m))
            attn_head(0, h, items)

        kv_stk.close()   # free wk/wv/xk/xv SBUF before phase 2

        # =====================================================================
        # qb 1..3 with interleaved norm/outproj of qb-1; then epilogue
        # =====================================================================
        with tc.tile_pool(name="pccp", bufs=1) as pccp, \
             tc.tile_pool(name="pwo", bufs=1) as pwo, \
             tc.tile_pool(name="pcch", bufs=4) as pcch, \
             tc.tile_pool(name="posb", bufs=3) as posb:

            ccp = [pccp.tile([b - a, S], BF16, tag=f"ccp{i}", name=f"ccp{i}")
                   for i, (a, b) in enumerate(PKT)]
            wop = []
            for i, (a, b) in enumerate(PKT):
                w = pwo.tile([b - a, D], BF16, tag=f"wop{i}", name=f"wop{i}")
                nc.sync.dma_start(w[:], wo_d[a:b, :])
                wop.append(w)
            sel8_t = pwo.tile([HPC, PC], BF16, tag="sel8")
            nc.sync.dma_start(sel8_t[:], sel8_d[:])
            selA_t = pwo.tile([4, PC], BF16, tag="selA")
            nc.sync.dma_start(selA_t[:], sel8_d[0:4, :])
            selB_t = pwo.tile([4, PC], BF16, tag="selB")
            nc.sync.dma_start(selB_t[:], sel8_d[4:HPC, :])

            def mk_items(qb, h):
                # Per-head pumped work. The reciprocal of qb-1 spreads over
                # heads 0-1 (it would saturate one head's DVE), norms of
                # qb-1 over heads 2-5, out-proj of qb-1 from head 6 with the
                # stragglers riding into qb+1; qb3 instead runs its own
                # heads-0-3 normalization early so the tail shrinks.
                q1 = qb - 1
                nrm = lambda hh: (lambda: norm1(q1, hh))
                op = lambda g, n: (lambda: outproj_half(g, n))
                rp = recip_items(rsall[q1 % 2], rcall[q1 % 2], f"u{q1}") \
                    if h == 0 else None
                if h == 0:
                    # stash parts 2,3 for head 1 (recip_items allocs once)
                    mk_items.rp23 = rp[1:]
                    it = [rp[0]]
                    if qb >= 2:
                        gm = (qb - 2) * 4 + 3
                        it += [op(gm, 0), op(gm, 1)]
                    return it
                if h == 1:
                    it = list(mk_items.rp23)
                elif h <= 5:
                    it = [nrm(2 * (h - 2)), nrm(2 * (h - 2) + 1)]
                else:
                    b0 = (h - 6) * 3   # halves 0..5 of blocks 0..2
                    it = [op(q1 * 4 + (b0 + k) // 2, (b0 + k) % 2)
                          for k in range(3)]
                if qb < 3:
                    if h == 1:
                        def _ld(c=qb + 1):
                            load_x_chunk("q", pxq, xq_d, c, "xq")
                            load_mask(c)
                        it.append(_ld)
                    if 2 <= h <= 5:
                        it.append(lambda c=qb + 1, m=h - 2: qchain(c, m))
                else:
                    if h == 5:
                        rpa = recip_items(rsA3, rcA3, "A3")
                        mk_items.rpa23 = rpa[1:]
                        it.append(rpa[0])
                    elif h == 6:
                        it = list(mk_items.rpa23) + [lambda: norm1(3, 0)]
                    elif h == 7:
                        it = [lambda: norm1(3, 1), lambda: norm1(3, 2),
                              lambda: norm1(3, 3)]
                return it

            for qb in range(1, QB):
                for h in range(HPC):
                    attn_head(qb, h, mk_items(qb, h))
            while pend:
                flush_one()

            # tail: the deferred qb2 out-proj blocks overlap the final
            # row-sum gather + reciprocal (emit them FIRST -- pool-slot
            # reuse waits on the issuing engine's full clock at emission,
            # so anything emitted after the recip would serialize on it);
            # then heads 4-7 of qb3 normalize and the last q-block
            # projects out.
            # tail: qb2's out-proj blocks act as PE filler around the final
            # row-sum reciprocal and heads 4-7 normalization of qb3
            for gm in (8, 9):
                outproj_half(gm, 0)
                outproj_half(gm, 1)
            recip_t(rsB3, rcB3, "B3", eng=nc.vector)
            outproj_half(10, 0)
            outproj_half(10, 1)
            for h in range(4, HPC):
                norm1(QB - 1, h)
            outproj_half(11, 0)
            outproj_half(11, 1)
            for gm in range((QB - 1) * 4, QB * 4):
                outproj_half(gm, 0)
                outproj_half(gm, 1)
        stk.close()

    return nc


def _prep_inputs(q, k, v, mask, Wq, bqv, Wk, bkv, Wv, bvv, Wo):
    """Per-core input maps (numpy, host-side shard + cast)."""
    in_maps = []
    sel8 = np.zeros((HPC, PC), np.float32)
    for h in range(HPC):
        sel8[h, h * DH:(h + 1) * DH] = 1.0
    sel8 = sel8.astype(BF)
    mask_h = {}
    for b in range(B):
        mt = (mask[b, 0] != 0).astype(np.float32).T  # [k, q]
        m4 = mt.reshape(KT, 128, QB, QW).transpose(2, 1, 0, 3)
        mask_h[b] = np.ascontiguousarray(m4.reshape(QB, 128, KT * QW)).astype(BF)
    for c in range(N_CORES):
        b, hh = c // 2, c % 2
        sl = slice(hh * PC, (hh + 1) * PC)
        Wk_l = Wk[sl, :]
        bq_l = bqv[sl]
        # k weights with one extra output column per head: Wk_h^T @ bq_h
        wkT_e = np.zeros((D, PCE), np.float32)
        for h in range(HPC):
            Wk_h = Wk_l[h * DH:(h + 1) * DH, :]
            wkT_e[:, h * DHE:h * DHE + DH] = Wk_h.T
            wkT_e[:, h * DHE + DH] = Wk_h.T @ bq_l[h * DH:(h + 1) * DH]
        in_maps.append({
            "xq": np.ascontiguousarray(q[b].T).astype(BF),
            "xk": np.ascontiguousarray(k[b].T).astype(BF),
            "xv": np.ascontiguousarray(v[b].T).astype(BF),
            "maskH": mask_h[b],
            "wqT": np.ascontiguousarray(Wq[sl, :].T).astype(BF),
            "wkT": wkT_e.astype(BF),
            "wvT": np.ascontiguousarray(Wv[sl, :].T).astype(BF),
            "woT": np.ascontiguousarray(Wo[:, sl].T).astype(BF),
            "sel8": sel8,
        })
    return in_maps


def run_sharded(in_maps, **kwargs):
    if "nc" not in _BUILT:
        _BUILT["nc"] = _build_nc()
    return run_bass_kernel_spmd(_BUILT["nc"], in_maps,
                                core_ids=list(range(N_CORES)), **kwargs)


def kernel(q, k, v, mask, Wq, bq, Wk, bk, Wv, bv, Wo, bo):
    q = np.asarray(q, np.float32)
    k = np.asarray(k, np.float32)
    v = np.asarray(v, np.float32)
    mask = np.asarray(mask)
    Wo32 = np.asarray(Wo, np.float32)
    bv32 = np.asarray(bv, np.float32)
    in_maps = _prep_inputs(q, k, v, mask,
                           np.asarray(Wq, np.float32), np.asarray(bq, np.float32),
                           np.asarray(Wk, np.float32), np.asarray(bk, np.float32),
                           np.asarray(Wv, np.float32), bv32, Wo32)
    res = run_sharded(in_maps)
    bo_eff = np.asarray(bo, np.float32) + bv32 @ Wo32.T
    out = np.empty((B, S, D), np.float32)
    for b in range(B):
        out[b] = res.results[2 * b]["out"] + res.results[2 * b + 1]["out"] + bo_eff
    return out


# revision 39
# speedup vs baseline: 1.0930x; 1.0187x over previous
"""Multi-head attention (B=4,S=2048,D=1024,H=16,dh=65) on 8 TRN2 NeuronCores.

Sharding: batch x head-half. Core c handles batch c//2 and heads
(c%2)*8..(c%2)*8+8 (P-slice of 520). Each core computes its QKV projections,
attention, and a partial out-projection; the host sums the two partials per
batch and adds bo.

v2: single fused pipeline. k/v projection chains are interleaved with the
first two heads' attention rounds of q-block 0; q-chains / normalization /
out-projection are pumped between rounds of later heads so the PE never
drains. Rounds are software-pipelined (scores of round r+1 are enqueued
before the AV matmuls of round r, so the PE does not sit behind exp/mask).

Bias handling (no bias matmuls at all):
  - scores((q+bq)(k+bk)) = q~k~ + bq.k~ + [q~.bk + bq.bk]; the bracketed
    terms are constant along the softmax axis (keys) and cancel in the
    normalized softmax, so they are dropped. bq.k~ is folded in as a 66th
    contraction row: the k-projection weight gains one output column per
    head (Wk_h^T bq_h) and the q side carries a constant-1 row.
  - bv enters the output as (bv @ Wo^T), a constant row folded into bo on
    the host.

Softmax runs unnormalized (score magnitudes are small); the row-sum is
harvested from a trailing ones-column in V and divided out at the end.
Compute dtype bf16 (fp32 PSUM accumulation).
"""

import math
import sys
from contextlib import ExitStack

import numpy as np
import ml_dtypes

sys.path.insert(0, "/opt/trn_rl_repo")

import concourse.bass as bass
import concourse.mybir as mybir
import concourse.tile as tile_mod
from concourse.bass_utils import run_bass_kernel_spmd
from concourse.masks import make_identity
from concourse.vector_clock import ScopedClock

# ---------------------------------------------------------------------------
# Patch for this container's walrus build: it rejects instructions carrying
# more than one semaphore wait ("Too many sync wait commands"), but Tile's
# wait assigner freely attaches several. Split excess waits onto bass_nofuse
# InstNoOp carriers on the same engine, committed immediately before the
# instruction (same-engine program order => over-synchronization only).
# ---------------------------------------------------------------------------
_MAX_WAITS = 1

_orig_commit = tile_mod.TileContext._commit_instruction


def _split_waits(self, inst, commit):
    si = inst.sync_info
    if si is None or len(si.on_wait) <= _MAX_WAITS:
        return
    waits = list(si.on_wait)
    sem_w = [w for w in waits if getattr(w, "sync_type", "semaphore") == "semaphore"]
    other_w = [w for w in waits if getattr(w, "sync_type", "semaphore") != "semaphore"]
    keep_budget = _MAX_WAITS - len(other_w)
    if keep_budget < 0:
        return
    keep = other_w + (sem_w[-keep_budget:] if keep_budget > 0 else [])
    excess = sem_w[: len(sem_w) - max(keep_budget, 0)]
    if not excess:
        return
    for i, w in enumerate(excess):
        nop = mybir.InstNoOp(
            name=f"{inst.name}-sw{i}",
            sync_info=mybir.SyncInfo(on_wait=[w], on_update=[]),
            bass_nofuse=True,
            engine=inst.engine,
        )
        commit(nop)
    inst.sync_info = mybir.SyncInfo(on_wait=keep, on_update=list(si.on_update))


def _patched_commit(self, inst, lazy_reg_writes: bool = True):
    if inst.engine != mybir.EngineType.Unassigned:
        _split_waits(self, inst, lambda n: _orig_commit(self, n, False))
    return _orig_commit(self, inst, lazy_reg_writes)


def _patched_drain_and_barrier(self, tick_clock, wait_clock):
    drain_inst = self.nc.sync.drain()
    wait_clock.add_sem_waits(
        drain_inst.ins, ScopedClock({None: tick_clock.global_clock})
    )
    si = drain_inst.ins.sync_info
    if si is not None and len(si.on_wait) > _MAX_WAITS:
        waits = list(si.on_wait)
        drain_inst.ins.sync_info = mybir.SyncInfo(
            on_wait=waits[:_MAX_WAITS], on_update=list(si.on_update)
        )
        for w in waits[_MAX_WAITS:]:
            n = self.nc.sync.nop(nofuse=True)
            n.ins.sync_info = mybir.SyncInfo(on_wait=[w], on_update=[])
    self.nc.all_engine_barrier()
    popped = self.nc._tile_sem_poison_stack.pop()
    assert popped is self._sem_poison
    self.nc.clear_and_free_semaphores(list(self.sems.allocated().values()))
    self.nc.all_engine_barrier()


tile_mod.TileContext._commit_instruction = _patched_commit
tile_mod.TileContext._drain_and_barrier = _patched_drain_and_barrier

# ---------------------------------------------------------------------------

B, S, D, H = 4, 2048, 1024, 16
DH = D // H + 1          # 65
DHE = DH + 1             # 66: +1 contraction row carrying bq.k~
P = H * DH               # 1040
HPC = H // 2             # 8 heads per core
PC = HPC * DH            # 520, per-core P slice
PCE = HPC * DHE          # 528, k-weight slice incl. bias-fold column
N_CORES = 8

MT = S // 128            # 16 row blocks / k tiles
KT = 16                  # k tiles per attention row
QB = 4                   # q blocks of 512
QW = 512
RKT = 2                  # k-tiles per score round
NR = KT // RKT           # 8 rounds per head

F32 = mybir.dt.float32
BF16 = mybir.dt.bfloat16
BF = ml_dtypes.bfloat16

# packed 128-row k-tile ranges of the 520-row concatT / WoT
PKT = [(0, 128), (128, 256), (256, 384), (384, 512), (512, 520)]

_BUILT = {}


def _build_nc():
    nc = bass.Bass("TRN2", target_bir_lowering=False, debug=False,
                   num_devices=N_CORES)

    xq_d = nc.dram_tensor("xq", [D, S], BF16, kind="ExternalInput").ap()
    xk_d = nc.dram_tensor("xk", [D, S], BF16, kind="ExternalInput").ap()
    xv_d = nc.dram_tensor("xv", [D, S], BF16, kind="ExternalInput").ap()
    # maskH[qb, p, j*QW+q] = maskT[j*128+p, qb*512+q] (multiplicative 0/1)
    mh = nc.dram_tensor("maskH", [QB, 128, KT * QW], BF16,
                        kind="ExternalInput").ap()
    wq_d = nc.dram_tensor("wqT", [D, PC], BF16, kind="ExternalInput").ap()
    wk_d = nc.dram_tensor("wkT", [D, PCE], BF16, kind="ExternalInput").ap()
    wv_d = nc.dram_tensor("wvT", [D, PC], BF16, kind="ExternalInput").ap()
    wo_d = nc.dram_tensor("woT", [PC, D], BF16, kind="ExternalInput").ap()
    sel8_d = nc.dram_tensor("sel8", [HPC, PC], BF16,
                            kind="ExternalInput").ap()
    out = nc.dram_tensor("out", [S, D], F32, kind="ExternalOutput").ap()

    inv_sqrt = 1.0 / math.sqrt(float(DH))

    with tile_mod.TileContext(nc) as tc:
        stk = ExitStack()
        pconst = stk.enter_context(tc.tile_pool(name="const", bufs=1))
        pkT = stk.enter_context(tc.tile_pool(name="pkT", bufs=1))
        pqT = stk.enter_context(tc.tile_pool(name="pqT", bufs=1))
        pvh = stk.enter_context(tc.tile_pool(name="pvh", bufs=1))
        prs = stk.enter_context(tc.tile_pool(name="prs", bufs=2))
        puov = stk.enter_context(tc.tile_pool(name="puov", bufs=10))
        ppt = stk.enter_context(tc.tile_pool(name="ppt", bufs=4))
        pmask = stk.enter_context(tc.tile_pool(name="pmask", bufs=3))
        prow = stk.enter_context(tc.tile_pool(name="prow", bufs=2))
        pwq = stk.enter_context(tc.tile_pool(name="pwq", bufs=1))
        pxq = stk.enter_context(tc.tile_pool(name="pxq", bufs=8))
        psS = stk.enter_context(tc.tile_pool(name="psS", bufs=2, space="PSUM"))
        psV = stk.enter_context(tc.tile_pool(name="psV", bufs=2, space="PSUM"))
        psA = stk.enter_context(tc.tile_pool(name="psA", bufs=2, space="PSUM"))

        # ---- persistent SBUF state ----------------------------------------
        ident = pconst.tile([128, 128], BF16, tag="ident")
        make_identity(nc, ident[:])

        kT = pkT.tile([DHE, HPC, S], BF16, tag="kT")
        qTs = [pqT.tile([DHE, HPC, QW], BF16, tag=f"qT{i}", name=f"qT{i}")
               for i in range(2)]
        for qt in qTs:
            # const-1 row at partition 65 (bias fold). Engine ops need a
            # 32-aligned partition base, so also touch row 64 -- every
            # q-chain overwrites rows 0..64 afterwards.
            nc.vector.memset(qt[64:DHE, :, :], 1.0)
        vh = [pvh.tile([128, HPC, DH + 1], BF16, tag=f"vh{j}", name=f"vh{j}")
              for j in range(MT)]
        for j in range(MT):
            nc.vector.memset(vh[j][:, :, DH:DH + 1], 1.0)  # row-sum column

        rsall = [prs.tile([HPC, QW], BF16, tag=f"rsall{i}", bufs=1,
                          name=f"rsall{i}") for i in range(2)]
        rcall = [prs.tile([HPC, QW], BF16, tag=f"rcall{i}", bufs=1,
                          name=f"rcall{i}") for i in range(2)]
        # split row-sum tiles for the last q-block: lets the tail overlap
        # normalization of heads 0-3 with heads 4-7's attention
        rsA3 = prs.tile([4, QW], BF16, tag="rsA3", bufs=1, name="rsA3")
        rsB3 = prs.tile([4, QW], BF16, tag="rsB3", bufs=1, name="rsB3")
        rcA3 = prs.tile([4, QW], BF16, tag="rcA3", bufs=1, name="rcA3")
        rcB3 = prs.tile([4, QW], BF16, tag="rcB3", bufs=1, name="rcB3")

        # ---- phase-0 weight / x pools (freed after qb0) --------------------
        kv_stk = ExitStack()
        pwkv = kv_stk.enter_context(tc.tile_pool(name="pwkv", bufs=1))
        pxk = kv_stk.enter_context(tc.tile_pool(name="pxk", bufs=12))
        pxv = kv_stk.enter_context(tc.tile_pool(name="pxv", bufs=12))

        uov_map = {}
        mask_tiles = {}
        xmap = {}

        def load_w(pool, dram, tag, de, eng=None):
            ts = []
            for d in range(8):
                t = pool.tile([128, HPC, de], BF16, tag=f"{tag}{d}",
                              bufs=1, name=f"{tag}{d}")
                (eng or nc.sync).dma_start(t[:], dram[d * 128:(d + 1) * 128, :])
                ts.append(t)
            return ts

        def load_x_chunk(which, pool, dram, c, tag, eng=None):
            ts = []
            for d in range(8):
                t = pool.tile([128, QW], BF16, tag=tag,
                              name=f"x{which}{c}_{d}")
                (eng or nc.sync).dma_start(
                    t[:], dram[d * 128:(d + 1) * 128, c * QW:(c + 1) * QW])
                ts.append(t)
            xmap[(which, c)] = ts

        def load_mask(qb):
            mts = []
            for hf in range(2):
                mt = pmask.tile([128, KT // 2, QW], BF16, tag="mask",
                                name=f"mask{qb}_{hf}")
                nc.sync.dma_start(
                    mt[:], mh[qb, :, hf * (KT // 2) * QW:
                              (hf + 1) * (KT // 2) * QW])
                mts.append(mt)
            mask_tiles[qb] = mts

        # ---- projection chains ---------------------------------------------
        def half_chain(which, m, half, wts, xts, de, qb=None):
            hs = half * 4
            co = (m % 4) * 128
            ps = psA.tile([128, 4, de], F32, tag="psA",
                          name=f"{which}ps{m}_{half}")
            for d in range(8):
                nc.tensor.matmul(ps[:], xts[d][:, co:co + 128],
                                 wts[d][:, hs:hs + 4, :],
                                 start=(d == 0), stop=(d == 7))
            if which == "v":
                nc.vector.tensor_copy(vh[m][:, hs:hs + 4, 0:DH], ps[:])
                return
            row = prow.tile([128, 4, DHE], BF16, tag=f"row{which}",
                            name=f"{which}row{m}_{half}")
            nc.vector.tensor_copy(row[:, :, 0:de], ps[:])
            pstr = psA.tile([128, 4, 128], BF16, tag="psA",
                            name=f"{which}pstr{m}_{half}")
            for t in range(4):
                nc.tensor.transpose(pstr[0:de, t, :], row[:, t, 0:de],
                                    ident[:])
            if which == "k":
                nc.vector.tensor_copy(
                    kT[0:de, hs:hs + 4, m * 128:(m + 1) * 128],
                    pstr[0:de, :, :])
            else:
                nc.scalar.copy(
                    qTs[qb % 2][0:de, hs:hs + 4, co:co + 128],
                    pstr[0:de, :, :])

        def kchain(m):
            for half in range(2):
                half_chain("k", m, half, wk_t, xmap[("k", m // 4)], DHE)

        def vchain(m):
            for half in range(2):
                half_chain("v", m, half, wv_t, xmap[("v", m // 4)], DH)

        def qchain(qb, m):
            for half in range(2):
                half_chain("q", m, half, wq_t, xmap[("q", qb)], DH, qb=qb)

        # ---- attention ------------------------------------------------------
        def mk_uov(qb, h, ov):
            uov = puov.tile([DH + 1, QW], BF16, tag="uov",
                            name=f"uov{qb}_{h}")
            nc.vector.tensor_copy(uov[:], ov[:])
            uov_map[(qb, h)] = uov
            if qb == QB - 1:
                # split (A/B) row-sum gathers so the tail recip/norm of
                # heads 0-3 can overlap heads 4-7's attention
                dst = rsA3 if h < 4 else rsB3
                nc.gpsimd.dma_start(dst[h % 4:h % 4 + 1, :],
                                    uov[DH:DH + 1, :])
            else:
                nc.gpsimd.dma_start(rsall[qb % 2][h:h + 1, :],
                                    uov[DH:DH + 1, :])

        # phase-1 (qb0 heads 0/1): single-round granularity, paced by the
        # k/v chains between rounds, AV lag of 1 round.
        def attn_round(qb, h, r):
            qt = qTs[qb % 2]
            ss = psS.tile([128, RKT, QW], F32, tag="psS",
                          name=f"ss{qb}_{h}_{r}")
            for jj in range(RKT):
                j = r * RKT + jj
                nc.tensor.matmul(ss[:, jj, :],
                                 kT[0:DHE, h, j * 128:(j + 1) * 128],
                                 qt[0:DHE, h, :], start=True, stop=True)
            pt = ppt.tile([128, RKT, QW], BF16, tag="pt1",
                          name=f"p1_{qb}_{h}_{r}", bufs=3)
            nc.scalar.activation(pt[:], ss[:],
                                 mybir.ActivationFunctionType.Exp,
                                 scale=inv_sqrt)
            mt = mask_tiles[qb][r // (NR // 2)]
            rr = r % (NR // 2)
            nc.vector.tensor_mul(pt[:], pt[:],
                                 mt[:, rr * RKT:(rr + 1) * RKT, :])
            return pt

        def emit_av(h, r, pt, ov):
            for jj in range(RKT):
                j = r * RKT + jj
                nc.tensor.matmul(ov[:], vh[j][:, h, :], pt[:, jj, :],
                                 start=(j == 0), stop=(j == KT - 1))

        # steady state: one continuous stream of (head, round-pair) units
        # across qb boundaries. exp per round (PSUM-bound), mask-mul per
        # round-PAIR (halves DVE op count), AV pairs lag 2 units so the
        # scores->exp->mask latency (~5us) never stalls the PE FIFO.
        pend = []

        def flush_one():
            qb_, h_, g_, ptS, ov = pend.pop(0)
            for jj4 in range(2 * RKT):
                j = g_ * 2 * RKT + jj4
                nc.tensor.matmul(ov[:], vh[j][:, h_, :], ptS[:, jj4, :],
                                 start=(j == 0), stop=(j == KT - 1))
            if g_ == NR // 2 - 1:
                mk_uov(qb_, h_, ov)

        def pair_unit(qb, h, g, ov):
            qt = qTs[qb % 2]
            ptS = ppt.tile([128, 2 * RKT, QW], BF16, tag="pt",
                           name=f"pt{qb}_{h}_{g}")
            for rr in range(2):
                r = 2 * g + rr
                ss = psS.tile([128, RKT, QW], F32, tag="psS",
                              name=f"ss{qb}_{h}_{r}")
                for jj in range(RKT):
                    j = r * RKT + jj
                    nc.tensor.matmul(ss[:, jj, :],
                                     kT[0:DHE, h, j * 128:(j + 1) * 128],
                                     qt[0:DHE, h, :], start=True, stop=True)
                nc.scalar.activation(ptS[:, rr * RKT:(rr + 1) * RKT, :],
                                     ss[:],
                                     mybir.ActivationFunctionType.Exp,
                                     scale=inv_sqrt)
            mt = mask_tiles[qb][g // 2]
            gg = g % 2
            nc.vector.tensor_mul(ptS[:], ptS[:],
                                 mt[:, gg * 2 * RKT:(gg + 1) * 2 * RKT, :])
            pend.append((qb, h, g, ptS, ov))
            if len(pend) > 2:
                flush_one()

        def attn_head(qb, h, items):
            ov = psV.tile([DH + 1, QW], F32, tag="ov", name=f"ov{qb}_{h}")
            for g in range(NR // 2):
                pair_unit(qb, h, g, ov)
                if g in (1, 2) and items:
                    items.pop(0)()
            while items:
                items.pop(0)()

        # ---- normalization + out-projection ---------------------------------
        def recip_t(src, dst, nm, eng=None):
            e = eng or nc.vector
            p = src.shape[0]
            rsf = prs.tile([p, QW], F32, tag=f"rsf{p}", name=f"rsf{nm}")
            e.tensor_copy(rsf[:], src[:])
            rcf = prs.tile([p, QW], F32, tag=f"rcf{p}", name=f"rcf{nm}")
            e.reciprocal(rcf[:], rsf[:])
            e.tensor_copy(dst[:], rcf[:])

        def recip_items(src, dst, nm):
            # the iterative reciprocal occupies the DVE for ~4us; split it
            # into three pump items so mask-multiplies interleave and the
            # exp stream never starves
            p = src.shape[0]
            rsf = prs.tile([p, QW], F32, tag=f"rsf{p}", name=f"rsf{nm}")
            rcf = prs.tile([p, QW], F32, tag=f"rcf{p}", name=f"rcf{nm}")

            def p1():
                nc.vector.tensor_copy(rsf[:], src[:])
                nc.vector.reciprocal(rcf[:, 0:QW // 2], rsf[:, 0:QW // 2])

            def p2():
                nc.vector.reciprocal(rcf[:, QW // 2:], rsf[:, QW // 2:])

            def p3():
                nc.vector.tensor_copy(dst[:], rcf[:])

            return [p1, p2, p3]

        def recip(qb):
            recip_t(rsall[qb % 2], rcall[qb % 2], f"u{qb}")

        def norm1(qb, h):
            rbp = psA.tile([128, QW], F32, tag="psA", name=f"rbp{qb}_{h}")
            if qb == QB - 1:
                sel, rc = (selA_t, rcA3) if h < 4 else (selB_t, rcB3)
                nc.tensor.matmul(rbp[0:DH, :], sel[:, h * DH:(h + 1) * DH],
                                 rc[:], start=True, stop=True)
            else:
                nc.tensor.matmul(rbp[0:DH, :],
                                 sel8_t[:, h * DH:(h + 1) * DH],
                                 rcall[qb % 2][:], start=True, stop=True)
            cch = pcch.tile([DH, QW], BF16, tag="cch", name=f"cch{qb}_{h}")
            nc.vector.tensor_mul(cch[:], rbp[0:DH, :],
                                 uov_map[(qb, h)][0:DH, :])
            r0 = h * DH
            for i, (a, b) in enumerate(PKT):
                lo, hi = max(r0, a), min(r0 + DH, b)
                if lo < hi:
                    # sync engine: keeps these off the gpsimd clock that
                    # the out-proj LDWEIGHTS conservatively waits on
                    nc.sync.dma_start(
                        ccp[i][lo - a:hi - a, qb * QW:(qb + 1) * QW],
                        cch[lo - r0:hi - r0, :])

        def outproj_half(gm, n):
            ps = psA.tile([128, QW], F32, tag="psA", name=f"pop{gm}_{n}")
            for i, (a, b) in enumerate(PKT):
                nc.tensor.matmul(ps[:], ccp[i][:, gm * 128:(gm + 1) * 128],
                                 wop[i][:, n * QW:(n + 1) * QW],
                                 start=(i == 0), stop=(i == len(PKT) - 1))
            osb = posb.tile([128, QW], F32, tag="osb", name=f"osb{gm}_{n}")
            nc.vector.tensor_copy(osb[:], ps[:])
            nc.sync.dma_start(
                out[gm * 128:(gm + 1) * 128, n * QW:(n + 1) * QW], osb[:])

        # =====================================================================
        # prologue: weight + first-chunk loads, q chunk-0 chains
        # =====================================================================
        # wq on sync, xq on gpsimd: the two DMA queues run in parallel so
        # the first q-chain starts ~7us in instead of ~16us
        wq_t = load_w(pwq, wq_d, "wq", DH)
        load_x_chunk("q", pxq, xq_d, 0, "xq", eng=nc.gpsimd)
        wk_t = load_w(pwkv, wk_d, "wk", DHE)
        load_x_chunk("k", pxk, xk_d, 0, "xk")
        wv_t = load_w(pwkv, wv_d, "wv", DH, eng=nc.gpsimd)
        load_x_chunk("v", pxv, xv_d, 0, "xv", eng=nc.gpsimd)
        load_mask(0)

        for m in range(4):
            qchain(0, m)

        # =====================================================================
        # qb0: k/v chains paced against heads 0,1; then heads 2..7
        # =====================================================================
        ov01 = [psV.tile([DH + 1, QW], F32, tag="ov", name=f"ov0_{h}")
                for h in range(2)]
        prev01 = [None, None]
        for r in range(NR):
            for m in (2 * r, 2 * r + 1):
                kchain(m)
                vchain(m)
                if m % 4 == 1 and m // 4 < 3:
                    load_x_chunk("k", pxk, xk_d, m // 4 + 1, "xk")
                    load_x_chunk("v", pxv, xv_d, m // 4 + 1, "xv")
            for h in range(2):
                pt = attn_round(0, h, r)
                if prev01[h] is not None:
                    emit_av(h, r - 1, prev01[h], ov01[h])
                prev01[h] = pt
        for h in range(2):
            emit_av(h, NR - 1, prev01[h], ov01[h])
            mk_uov(0, h, ov01[h])

        for h in range(2, HPC):
            items = []
            if h == 2:
                def _ld1():
                    load_x_chunk("q", pxq, xq_d, 1, "xq")
                    load_mask(1)
                items.append(_ld1)
            if 3 <= h <= 6:
                items.append(lambda m=h - 3: qchain(1, m))
            attn_head(0, h, items)

        kv_stk.close()   # free wk/wv/xk/xv SBUF before phase 2

        # =====================================================================
        # qb 1..3 with interleaved norm/outproj of qb-1; then epilogue
        # =====================================================================
        with tc.tile_pool(name="pccp", bufs=1) as pccp, \
             tc.tile_pool(name="pwo", bufs=1) as pwo, \
             tc.tile_pool(name="pcch", bufs=3) as pcch, \
             tc.tile_pool(name="posb", bufs=4) as posb:

            ccp = [pccp.tile([b - a, S], BF16, tag=f"ccp{i}", name=f"ccp{i}")
                   for i, (a, b) in enumerate(PKT)]
            wop = []
            for i, (a, b) in enumerate(PKT):
                w = pwo.tile([b - a, D], BF16, tag=f"wop{i}", name=f"wop{i}")
                nc.sync.dma_start(w[:], wo_d[a:b, :])
                wop.append(w)
            sel8_t = pwo.tile([HPC, PC], BF16, tag="sel8")
            nc.sync.dma_start(sel8_t[:], sel8_d[:])
            selA_t = pwo.tile([4, PC], BF16, tag="selA")
            nc.sync.dma_start(selA_t[:], sel8_d[0:4, :])
            selB_t = pwo.tile([4, PC], BF16, tag="selB")
            nc.sync.dma_start(selB_t[:], sel8_d[4:HPC, :])

            def mk_items(qb, h):
                # Per-head pumped work. The reciprocal of qb-1 spreads over
                # heads 0-1 (it would saturate one head's DVE), norms of
                # qb-1 over heads 2-5, out-proj of qb-1 from head 6 with the
                # stragglers riding into qb+1; qb3 instead runs its own
                # heads-0-3 normalization early so the tail shrinks.
                q1 = qb - 1
                nrm = lambda hh: (lambda: norm1(q1, hh))
                op = lambda g, n: (lambda: outproj_half(g, n))
                rp = recip_items(rsall[q1 % 2], rcall[q1 % 2], f"u{q1}") \
                    if h == 0 else None
                if h == 0:
                    # stash parts 2,3 for head 1 (recip_items allocs once)
                    mk_items.rp23 = rp[1:]
                    it = [rp[0]]
                    if qb >= 2:
                        gm = (qb - 2) * 4 + 3
                        it += [op(gm, 0), op(gm, 1)]
                    return it
                if h == 1:
                    it = list(mk_items.rp23)
                elif h <= 5:
                    it = [nrm(2 * (h - 2)), nrm(2 * (h - 2) + 1)]
                else:
                    b0 = (h - 6) * 3   # halves 0..5 of blocks 0..2
                    it = [op(q1 * 4 + (b0 + k) // 2, (b0 + k) % 2)
                          for k in range(3)]
                if qb < 3:
                    if h == 1:
                        def _ld(c=qb + 1):
                            load_x_chunk("q", pxq, xq_d, c, "xq")
                            load_mask(c)
                        it.append(_ld)
                    if 2 <= h <= 5:
                        it.append(lambda c=qb + 1, m=h - 2: qchain(c, m))
                else:
                    if h == 5:
                        rpa = recip_items(rsA3, rcA3, "A3")
                        mk_items.rpa23 = rpa[1:]
                        it.append(rpa[0])
                    elif h == 6:
                        it = list(mk_items.rpa23) + [lambda: norm1(3, 0)]
                    elif h == 7:
                        it = [lambda: norm1(3, 1), lambda: norm1(3, 2),
                              lambda: norm1(3, 3)]
                return it

            for qb in range(1, QB):
                for h in range(HPC):
                    attn_head(qb, h, mk_items(qb, h))
            while pend:
                flush_one()

            # tail: the deferred qb2 out-proj blocks overlap the final
            # row-sum gather + reciprocal (emit them FIRST -- pool-slot
            # reuse waits on the issuing engine's full clock at emission,
            # so anything emitted after the recip would serialize on it);
            # then heads 4-7 of qb3 normalize and the last q-block
            # projects out.
            # tail: qb2's out-proj blocks act as PE filler around the final
            # row-sum reciprocal and heads 4-7 normalization of qb3
            for gm in (8, 9, 10):
                outproj_half(gm, 0)
                outproj_half(gm, 1)
            recip_t(rsB3, rcB3, "B3", eng=nc.vector)
            for h in range(4, HPC):
                norm1(QB - 1, h)
            outproj_half(11, 0)
            outproj_half(11, 1)
            for gm in range((QB - 1) * 4, QB * 4):
                outproj_half(gm, 0)
                outproj_half(gm, 1)
        stk.close()

    return nc


def _prep_inputs(q, k, v, mask, Wq, bqv, Wk, bkv, Wv, bvv, Wo):
    """Per-core input maps (numpy, host-side shard + cast)."""
    in_maps = []
    sel8 = np.zeros((HPC, PC), np.float32)
    for h in range(HPC):
        sel8[h, h * DH:(h + 1) * DH] = 1.0
    sel8 = sel8.astype(BF)
    mask_h = {}
    for b in range(B):
        mt = (mask[b, 0] != 0).astype(np.float32).T  # [k, q]
        m4 = mt.reshape(KT, 128, QB, QW).transpose(2, 1, 0, 3)
        mask_h[b] = np.ascontiguousarray(m4.reshape(QB, 128, KT * QW)).astype(BF)
    for c in range(N_CORES):
        b, hh = c // 2, c % 2
        sl = slice(hh * PC, (hh + 1) * PC)
        Wk_l = Wk[sl, :]
        bq_l = bqv[sl]
        # k weights with one extra output column per head: Wk_h^T @ bq_h
        wkT_e = np.zeros((D, PCE), np.float32)
        for h in range(HPC):
            Wk_h = Wk_l[h * DH:(h + 1) * DH, :]
            wkT_e[:, h * DHE:h * DHE + DH] = Wk_h.T
            wkT_e[:, h * DHE + DH] = Wk_h.T @ bq_l[h * DH:(h + 1) * DH]
        in_maps.append({
            "xq": np.ascontiguousarray(q[b].T).astype(BF),
            "xk": np.ascontiguousarray(k[b].T).astype(BF),
            "xv": np.ascontiguousarray(v[b].T).astype(BF),
            "maskH": mask_h[b],
            "wqT": np.ascontiguousarray(Wq[sl, :].T).astype(BF),
            "wkT": wkT_e.astype(BF),
            "wvT": np.ascontiguousarray(Wv[sl, :].T).astype(BF),
            "woT": np.ascontiguousarray(Wo[:, sl].T).astype(BF),
            "sel8": sel8,
        })
    return in_maps


def run_sharded(in_maps, **kwargs):
    if "nc" not in _BUILT:
        _BUILT["nc"] = _build_nc()
    return run_bass_kernel_spmd(_BUILT["nc"], in_maps,
                                core_ids=list(range(N_CORES)), **kwargs)


def kernel(q, k, v, mask, Wq, bq, Wk, bk, Wv, bv, Wo, bo):
    q = np.asarray(q, np.float32)
    k = np.asarray(k, np.float32)
    v = np.asarray(v, np.float32)
    mask = np.asarray(mask)
    Wo32 = np.asarray(Wo, np.float32)
    bv32 = np.asarray(bv, np.float32)
    in_maps = _prep_inputs(q, k, v, mask,
                           np.asarray(Wq, np.float32), np.asarray(bq, np.float32),
                           np.asarray(Wk, np.float32), np.asarray(bk, np.float32),
                           np.asarray(Wv, np.float32), bv32, Wo32)
    res = run_sharded(in_maps)
    bo_eff = np.asarray(bo, np.float32) + bv32 @ Wo32.T
    out = np.empty((B, S, D), np.float32)
    for b in range(B):
        out[b] = res.results[2 * b]["out"] + res.results[2 * b + 1]["out"] + bo_eff
    return out
